# revision 5
# baseline (speedup 1.0000x reference)
"""ClusterMemory forward loss on 8 Trainium2 NeuronCores.

loss = -mean_b[ log_softmax(inputs @ features.T / TEMP)[b, targets[b]] ]
  inputs   [64, 2048] f32 (L2-normalized rows)
  targets  [64] int
  features [65536, 2048] f32 (L2-normalized rows)

Sharding: the feature bank is split row-wise across 8 cores (8192 rows each,
64 MB/core -> memory-bound streaming). Each core computes its partial logits
x @ f_shard.T (pre-scaled by 1/TEMP) and reduces them on-chip to per-512-column
(-max, sum exp(l - max)) pairs. The host combines the 8*16 partial (max, sumexp)
stats with a logsumexp tree and adds the target logit (64 dot products).

Device layout per core:
  xs [128, 16, 64] f32  : xs[p, k, b] = inputs[b, 128k+p] / TEMP  (replicated)
  fT [2048, 8192] f32   : fT[d, j] = features[8192c + j, d]       (sharded)
  out stats [64, 32] f32: stats[b, 2g] = -max_j logits[b, 512g+j]
                          stats[b, 2g+1] = sum_j exp(logits - max)
"""

import numpy as np

B = 64
N = 65536
D = 2048
TEMP = 0.05
NCORES = 8
SHARD = N // NCORES        # 8192 feature rows per core
KP = 128                   # contraction tile (SBUF partitions)
KTILES = D // KP           # 16
GROUP = 512                # psum free-dim per stats group
SUPER = 2048               # columns per DMA chunk / supergroup
GPS = SUPER // GROUP       # 4 psum groups per supergroup
NSUPER = SHARD // SUPER    # 4
NGROUPS = SHARD // GROUP   # 16
FBUFS = 12                 # prefetch depth for feature tiles (12 MB SBUF)
REPEATS = 1                # full streaming passes (>1 only for benchmarking)


def _hoist_extra_waits(nc, max_waits=1):
    """walrus in this container rejects >1 sync-wait command on most
    instruction encodings (Drain, LDWEIGHTS, ...). Hoist all but the last
    wait of every instruction onto standalone EventSemaphore instructions
    inserted just before it in the same engine's stream — semantically
    identical (the engine blocks on each in order)."""
    from concourse import mybir

    idx = 0
    for fn in nc.m.functions:
        for b in fn.blocks:
            out = []
            changed = False
            for ins in b.instructions:
                si = getattr(ins, "sync_info", None)
                if si is not None and len(si.on_wait) > max_waits:
                    waits = list(si.on_wait)
                    for w in waits[:-max_waits]:
                        idx += 1
                        e = mybir.InstEventSemaphore(
                            name=f"hoistw-{idx}", engine=ins.engine
                        )
                        e.sync_info = mybir.SyncInfo(on_wait=[w], on_update=[])
                        out.append(e)
                    ins.sync_info = mybir.SyncInfo(
                        on_wait=waits[-max_waits:], on_update=list(si.on_update)
                    )
                    changed = True
                out.append(ins)
            if changed:
                b.instructions = out
    return nc


def build_nc(repeats: int = REPEATS, hoist: bool = True):
    """Build the per-core Bass module (identical on all 8 cores)."""
    import concourse.bass as bass
    import concourse.tile as tile
    from concourse import mybir

    f32 = mybir.dt.float32
    nc = bass.Bass()
    xs = nc.dram_tensor("xs", [KP, KTILES, B], f32, kind="ExternalInput")
    fT = nc.dram_tensor("fT", [D, SHARD], f32, kind="ExternalInput")
    stats = nc.dram_tensor("stats", [B, 2 * NGROUPS], f32, kind="ExternalOutput")

    with tile.TileContext(nc) as tc:
        import contextlib

        with contextlib.ExitStack() as ctx:
            singles = ctx.enter_context(tc.tile_pool(name="singles", bufs=1))
            fpool = ctx.enter_context(tc.tile_pool(name="fpool", bufs=FBUFS))
            ppool = ctx.enter_context(
                tc.tile_pool(name="ppool", bufs=2 * GPS, space="PSUM")
            )
            epool = ctx.enter_context(tc.tile_pool(name="epool", bufs=3))

            xs_sb = singles.tile([KP, KTILES, B], f32)
            nc.sync.dma_start(xs_sb[:], xs[:])
            stats_sb = singles.tile([B, 2 * NGROUPS], f32)

            for _ in range(repeats):
                for J in range(NSUPER):
                    psums = [
                        ppool.tile([B, GROUP], f32, tag="ps", name=f"ps{J}_{jj}")
                        for jj in range(GPS)
                    ]
                    for k in range(KTILES):
                        ft = fpool.tile([KP, SUPER], f32, tag="ft")
                        nc.sync.dma_start(
                            ft[:],
                            fT[k * KP : (k + 1) * KP, J * SUPER : (J + 1) * SUPER],
                        )
                        for jj in range(GPS):
                            nc.tensor.matmul(
                                psums[jj][:],
                                xs_sb[:, k, :],
                                ft[:, jj * GROUP : (jj + 1) * GROUP],
                                start=(k == 0),
                                stop=(k == KTILES - 1),
                            )
                    for jj in range(GPS):
                        g = GPS * J + jj
                        nc.vector.reduce_max(
                            stats_sb[:, 2 * g : 2 * g + 1],
                            psums[jj][:],
                            axis=mybir.AxisListType.X,
                            negate=True,
                        )
                        et = epool.tile([B, GROUP], f32, tag="et")
                        nc.scalar.activation(
                            et[:],
                            psums[jj][:],
                            mybir.ActivationFunctionType.Exp,
                            bias=stats_sb[:, 2 * g : 2 * g + 1],
                            scale=1.0,
                            accum_out=stats_sb[:, 2 * g + 1 : 2 * g + 2],
                        )
            nc.sync.dma_start(stats[:], stats_sb[:])
    return _hoist_extra_waits(nc) if hoist else nc


def prep_inputs(inputs, features):
    """Host-side shard/layout prep shared by kernel() and test harnesses."""
    x32 = np.ascontiguousarray(np.asarray(inputs, dtype=np.float32))
    f32v = np.asarray(features, dtype=np.float32)
    xscaled = x32 / np.float32(TEMP)
    xs = np.ascontiguousarray(
        xscaled.T.reshape(KTILES, KP, B).transpose(1, 0, 2)
    )  # [128, 16, 64]
    in_maps = []
    for c in range(NCORES):
        fT_c = np.ascontiguousarray(f32v[c * SHARD : (c + 1) * SHARD].T)
        in_maps.append({"xs": xs, "fT": fT_c})
    return x32, f32v, in_maps


def combine(stats_list, x32, f32v, targets):
    """Host logsumexp combine of per-core stats + target logits -> loss."""
    neg_m = np.stack([s[:, 0::2] for s in stats_list], axis=1)  # [B, C, G]
    s_sum = np.stack([s[:, 1::2] for s in stats_list], axis=1)  # [B, C, G]
    m = (-neg_m).reshape(B, -1).astype(np.float64)
    s = s_sum.reshape(B, -1).astype(np.float64)
    M = m.max(axis=1)
    S = (s * np.exp(m - M[:, None])).sum(axis=1)
    logZ = M + np.log(S)
    tgt = np.asarray(targets).astype(np.int64)
    t = (x32.astype(np.float64) * f32v[tgt].astype(np.float64)).sum(axis=1) / TEMP
    loss = -(t - logZ).mean()
    return np.array(loss, dtype=np.float32)


def kernel(inputs, targets, features):
    from concourse.bass_utils import run_bass_kernel_spmd

    x32, f32v, in_maps = prep_inputs(inputs, features)
    nc = build_nc()
    res = run_bass_kernel_spmd(nc, in_maps, core_ids=list(range(NCORES)))
    stats_list = [res.results[c]["stats"] for c in range(NCORES)]
    return combine(stats_list, x32, f32v, targets)
